# revision 29
# baseline (speedup 1.0000x reference)
"""FP8 GEMM kernel for Trainium2 (8 NeuronCores, SPMD data-parallel over tokens).

Computes: out = fp16( fp32( e5m2(x) @ e4m3(weight.T) ) + bias )
  x      [4, 4096, 4096] fp16
  weight [4096, 4096]    fp16  (out_features, in_features)
  bias   [4096]          fp16
  out    [4, 4096, 4096] fp16

Sharding: token dim (B*S = 16384) split across 8 cores (2048 rows each);
weight + bias replicated. No collectives; host concatenates the outputs.

Layout: the host pre-packs both operands into per-tile K-major blocks
(`[tile][ki=128][ko=32][free]`) so every device load is fully contiguous.

Per-core pipeline (~478us vs the 442us PE floor of 2048 DoubleRow fp8
matmuls x 216ns; HBM read is ~390GB/s SHARED across HWDGE+SWDGE with
strict HWDGE priority, so the ramp rations it carefully):
 - Only x0,x1 arrive as raw fp16 on the sync HWDGE ring (during SWDGE's
   ~4us startup dead-zone) and are cast to e5m2 on DVE (bit-exact RNE,
   probed vs ml_dtypes). The ring then goes silent: any extra HWDGE
   traffic in the first ~100us preempts the weight stream and starves
   the PE (measured +25us).
 - Everything else is a single SWDGE cast-DMA FIFO in exact consumption
   order: x2,x3 first halves prefetched ahead of w0 so the col-0 wave
   can be 4-wide, weights ko-sliced for the early columns (w0 8x, w1 4x,
   w2 2x) alternating with x tiles at the work-per-byte-optimal ratio.
 - The PE schedule is a sequence of waves matched to slice arrival:
   wave(col0; m0..3) rides w0 quads at exactly the arrival rate (zero
   stalls, which also keeps the PE out of its low p-state), wave(col1;
   m0..3) rides w1 slices, wave(col2; m0..5) rides w2 halves; between
   waves, freshly landed x tiles are consumed against all resident
   columns. After the ~55us saturation point the PE never stalls again
   (w-pool depth 5 so no weight load waits on a buffer it can't have).
 - Bias add fused into the PSUM eviction on DVE; bias + stores ride the
   scalar (Act) HWDGE ring so they never queue behind loads. The last
   group is evicted in two 256-wide halves to shave the exposed tail.
"""

import sys

if "/opt/trn_rl_repo" not in sys.path:
    sys.path.insert(0, "/opt/trn_rl_repo")

import numpy as np

B, S, DIN, DOUT = 4, 4096, 4096, 4096
NCORES = 8
M_TOTAL = B * S              # 16384
M_LOC = M_TOTAL // NCORES    # 2048
P = 128
M_TILES = M_LOC // P         # 16 m-tiles of 128 rows
N_TILE = 512
N_TILES = DOUT // N_TILE     # 8
K_SUB = DIN // P             # 32 k-subtiles of 128
K_CHUNKS = K_SUB // 2        # 16 DoubleRow chunks of 256

_cached_nc = None


def _build():
    global _cached_nc
    if _cached_nc is not None:
        return _cached_nc

    import concourse.mybir as mybir
    import concourse.tile as tile
    from concourse import bacc

    nc = bacc.Bacc("TRN2", target_bir_lowering=False, debug=False,
                   num_devices=NCORES)

    # host-packed K-major tile blocks (see make_in_maps)
    xd = nc.dram_tensor("xd", [M_TILES, P, K_SUB, P], mybir.dt.float16,
                        kind="ExternalInput")
    wd = nc.dram_tensor("wd", [N_TILES, P, K_SUB, N_TILE], mybir.dt.float16,
                        kind="ExternalInput")
    bvec = nc.dram_tensor("bvec", [DOUT], mybir.dt.float16,
                          kind="ExternalInput")
    out = nc.dram_tensor("out", [M_LOC, DOUT], mybir.dt.float16,
                         kind="ExternalOutput")

    with tile.TileContext(nc) as tc:
        with tc.tile_pool(name="w8p", bufs=5) as w8p, \
             tc.tile_pool(name="x8p", bufs=1) as x8p, \
             tc.tile_pool(name="xst", bufs=2) as stp, \
             tc.tile_pool(name="outp", bufs=8) as outp, \
             tc.tile_pool(name="cst", bufs=1) as cst, \
             tc.tile_pool(name="psum", bufs=8, space="PSUM") as psump:

            # resident fp8 x: 16 tiles of [ki, ko, 128] e5m2
            x8 = [x8p.tile([P, K_SUB, P], mybir.dt.float8e5,
                           tag=f"x8_{m}", name=f"x8_{m}")
                  for m in range(M_TILES)]
            xst = {}
            w8 = {}

            # ---- scalar (Act) HWDGE ring: bias broadcast first ----
            bias_rep = cst.tile([P, DOUT], mybir.dt.float16)
            nc.scalar.dma_start(bias_rep[:],
                                bvec.ap()[None, :].to_broadcast((P, DOUT)))

            def load_x_hw(m, slices, eng):
                xst[m] = stp.tile([P, K_SUB, P], mybir.dt.float16,
                                  tag="xst", name=f"xst_{m}")
                return [eng.dma_start(xst[m][:, a:b, :], xd[m, :, a:b, :])
                        for a, b in slices]

            def cast_x(m, slices):
                return [nc.vector.tensor_copy(x8[m][:, a:b, :],
                                              xst[m][:, a:b, :])
                        for a, b in slices]

            # ---- sync HWDGE ring: x0,x1 plus the first halves of x2,x3 as
            # raw fp16 (the two DGE paths warm up in parallel with separate
            # curves, so this balances the ramp against SWDGE-carried w0);
            # the ring then goes silent so it never preempts the weight
            # stream (HWDGE has strict priority) ----
            xst[0] = stp.tile([P, K_SUB, P], mybir.dt.float16,
                              tag="xst", name="xst_0")
            xst[1] = stp.tile([P, K_SUB, P], mybir.dt.float16,
                              tag="xst", name="xst_1")
            xh2 = stp.tile([P, K_SUB // 2, P], mybir.dt.float16,
                           tag="xsth", name="xh2")
            xh3 = stp.tile([P, K_SUB // 2, P], mybir.dt.float16,
                           tag="xsth", name="xh3")
            nc.sync.dma_start(xst[0][:, 0:8, :], xd[0, :, 0:8, :])
            nc.sync.dma_start(xst[0][:, 8:32, :], xd[0, :, 8:32, :])
            nc.sync.dma_start(xst[1][:, 0:16, :], xd[1, :, 0:16, :])
            nc.sync.dma_start(xh2[:], xd[2, :, 0:16, :])
            nc.sync.dma_start(xh3[:], xd[3, :, 0:16, :])
            nc.sync.dma_start(xst[1][:, 16:32, :], xd[1, :, 16:32, :])
            cast_x(0, [(0, 4), (4, 8), (8, 20), (20, 32)])
            nc.vector.tensor_copy(x8[1][:, 0:16, :], xst[1][:, 0:16, :])
            nc.vector.tensor_copy(x8[2][:, 0:16, :], xh2[:])
            nc.vector.tensor_copy(x8[3][:, 0:16, :], xh3[:])
            nc.vector.tensor_copy(x8[1][:, 16:32, :], xst[1][:, 16:32, :])

            # ---- gpsimd SWDGE: single FIFO, greedy x:w balance (take
            # another x tile while m < 4*cols, else the next w column) ----
            def load_w(j, chunks):
                w8[j] = w8p.tile([P, K_SUB, N_TILE], mybir.dt.float8e4,
                                 tag="w8", name=f"w8_{j}")
                step = K_SUB // chunks
                for c in range(chunks):
                    ko = slice(c * step, (c + 1) * step)
                    nc.gpsimd.dma_start(w8[j][:, ko, :], wd[j, :, ko, :])

            def load_x_sw(m):
                nc.gpsimd.dma_start(x8[m][:], xd[m, :, :, :])

            w8[0] = w8p.tile([P, K_SUB, N_TILE], mybir.dt.float8e4,
                             tag="w8", name="w8_0")
            w8[1] = w8p.tile([P, K_SUB, N_TILE], mybir.dt.float8e4,
                             tag="w8", name="w8_1")

            def load_w_slice(j, a, b):
                return nc.gpsimd.dma_start(w8[j][:, a:b, :],
                                           wd[j, :, a:b, :])

            def load_x_slice(m, a, b):
                return nc.gpsimd.dma_start(x8[m][:, a:b, :],
                                           xd[m, :, a:b, :])

            for c in range(4):
                load_w_slice(0, 4 * c, 4 * c + 4)
            load_x_slice(2, 16, 32)
            load_x_slice(3, 16, 32)
            load_x_sw(4)
            for c in range(4, 8):
                load_w_slice(0, 4 * c, 4 * c + 4)
            load_w_slice(1, 0, 8)
            load_w_slice(1, 8, 16)
            load_x_sw(5)
            load_w_slice(1, 16, 24)
            load_w_slice(1, 24, 32)
            load_x_sw(6)
            load_x_sw(7)
            load_w(2, chunks=2)
            load_x_sw(8)
            load_x_sw(9)
            load_w(3, chunks=1)
            load_x_sw(10)
            load_x_sw(11)
            load_w(4, chunks=1)
            load_x_sw(12)
            load_x_sw(13)
            load_w(5, chunks=1)
            load_x_sw(14)
            load_x_sw(15)
            load_w(6, chunks=1)
            load_w(7, chunks=1)

            def mm(ps, j, m, kc):
                nc.tensor.matmul(
                    ps[:],
                    x8[m][:, 2 * kc:2 * kc + 2, :],
                    w8[j][:, 2 * kc:2 * kc + 2, :],
                    start=(kc == 0),
                    stop=(kc == K_CHUNKS - 1),
                    perf_mode=mybir.MatmulPerfMode.DoubleRow,
                )

            def evict(j, m, ps, split=False):
                ns = slice(j * N_TILE, (j + 1) * N_TILE)
                rows = slice(m * P, (m + 1) * P)
                if not split:
                    ob = outp.tile([P, N_TILE], mybir.dt.float16, tag="ob",
                                   name=f"ob_{j}_{m}")
                    nc.vector.tensor_add(ob[:], ps[:], bias_rep[:, ns])
                    nc.scalar.dma_start(out[rows, ns], ob[:])
                else:
                    h = N_TILE // 2
                    for half in range(2):
                        cs = slice(half * h, (half + 1) * h)
                        on = slice(j * N_TILE + half * h,
                                   j * N_TILE + (half + 1) * h)
                        ob = outp.tile([P, h], mybir.dt.float16, tag="obh",
                                       name=f"obh_{half}")
                        nc.vector.tensor_add(ob[:], ps[:, cs],
                                             bias_rep[:, on])
                        nc.scalar.dma_start(out[rows, on], ob[:])

            def do_group(j, m, split=False):
                ps = psump.tile([P, N_TILE], mybir.dt.float32, tag="ps",
                                name=f"ps_{j}_{m}")
                for kc in range(K_CHUNKS):
                    mm(ps, j, m, kc)
                evict(j, m, ps, split=split)

            def wave(j, ms, kc_run):
                """Interleaved chains over `ms`, kc emitted in runs of
                `kc_run` so each arriving w-slice unlocks a full-width run."""
                pss = {m: psump.tile([P, N_TILE], mybir.dt.float32, tag="ps",
                                     name=f"ps_{j}_{m}") for m in ms}
                for r0 in range(0, K_CHUNKS, kc_run):
                    for kc in range(r0, r0 + kc_run):
                        for m in ms:
                            mm(pss[m], j, m, kc)
                return pss

            # ---- phase A: col-0 wave over m0..3 riding w0 quads (x2,x3
            # prefetched ahead of w0, so consumption rate matches arrival) --
            psA = wave(0, [0, 1, 2, 3], kc_run=2)
            for m in range(4):
                evict(0, m, psA[m])

            # ---- phase B: (0,4) as x4 lands mid-w0 ----
            do_group(0, 4)

            # ---- phase C: col-1 wave over m0..3 riding w1 slices ----
            psC = wave(1, [0, 1, 2, 3], kc_run=4)
            for m in range(4):
                evict(1, m, psC[m])

            # ---- phase D: m4,m5 catch-up against cols 0,1 ----
            do_group(0, 5)
            do_group(1, 4)
            do_group(1, 5)

            # ---- phase E: col-2 wave over m0..5 riding w2 halves ----
            psE = wave(2, list(range(6)), kc_run=8)
            for m in range(6):
                evict(2, m, psE[m])

            # ---- phase F: m6,m7 against cols 0,1,2 ----
            for m in (6, 7):
                for j in range(3):
                    do_group(j, m)

            # ---- phase G: col-3 m0..7 (w3 fully resident by now) ----
            for m in range(8):
                do_group(3, m)

            # ---- phase H: m8..15 against cols 0..3 ----
            for m in range(8, M_TILES):
                for j in range(4):
                    do_group(j, m)

            # ---- phase I: columns 4..7 ----
            for j in range(4, N_TILES):
                for m in range(M_TILES):
                    last = (j == N_TILES - 1 and m == M_TILES - 1)
                    do_group(j, m, split=last)

    nc.compile()
    _cached_nc = nc
    return nc


def make_in_maps(x, weight, bias):
    x = np.asarray(x)
    weight = np.asarray(weight)
    bias = np.ascontiguousarray(np.asarray(bias))
    assert x.dtype == np.float16 and weight.dtype == np.float16

    # weight [DOUT, DIN] -> [j, ki, ko, n]: wd[j,ki,ko,n] = weight[j*512+n,
    # ko*128+ki] (i.e. weight.T in per-tile K-major blocks)
    wd = np.ascontiguousarray(
        weight.reshape(N_TILES, N_TILE, K_SUB, P).transpose(0, 3, 2, 1))

    xf = x.reshape(M_TOTAL, DIN)
    in_maps = []
    for c in range(NCORES):
        xc = xf[c * M_LOC:(c + 1) * M_LOC]
        # [M_LOC, DIN] -> [m-tile, ki, ko, m]: xd[t,ki,ko,m] = xc[t*128+m,
        # ko*128+ki]
        xd = np.ascontiguousarray(
            xc.reshape(M_TILES, P, K_SUB, P).transpose(0, 3, 2, 1))
        in_maps.append({"xd": xd, "wd": wd, "bvec": bias})
    return in_maps


def gather_out(results):
    out = np.concatenate([r["out"] for r in results], axis=0)
    return out.reshape(B, S, DOUT)


def kernel(x, weight, bias):
    from concourse.bass_utils import run_bass_kernel_spmd

    nc = _build()
    in_maps = make_in_maps(x, weight, bias)
    res = run_bass_kernel_spmd(nc, in_maps, core_ids=list(range(NCORES)))
    return gather_out(res.results)


# revision 31
# speedup vs baseline: 1.0006x; 1.0006x over previous
"""FP8 GEMM kernel for Trainium2 (8 NeuronCores, SPMD data-parallel over tokens).

Computes: out = fp16( fp32( e5m2(x) @ e4m3(weight.T) ) + bias )
  x      [4, 4096, 4096] fp16
  weight [4096, 4096]    fp16  (out_features, in_features)
  bias   [4096]          fp16
  out    [4, 4096, 4096] fp16

Sharding: token dim (B*S = 16384) split across 8 cores (2048 rows each);
weight + bias replicated. No collectives; host concatenates the outputs.

Layout: the host pre-packs both operands into per-tile K-major blocks
(`[tile][ki=128][ko=32][free]`) so every device load is fully contiguous.

Per-core pipeline (~478us vs the 442us PE floor of 2048 DoubleRow fp8
matmuls x 216ns; HBM read is ~390GB/s SHARED across HWDGE+SWDGE with
strict HWDGE priority, so the ramp rations it carefully):
 - Only x0,x1 arrive as raw fp16 on the sync HWDGE ring (during SWDGE's
   ~4us startup dead-zone) and are cast to e5m2 on DVE (bit-exact RNE,
   probed vs ml_dtypes). The ring then goes silent: any extra HWDGE
   traffic in the first ~100us preempts the weight stream and starves
   the PE (measured +25us).
 - Everything else is a single SWDGE cast-DMA FIFO in exact consumption
   order: x2,x3 first halves prefetched ahead of w0 so the col-0 wave
   can be 4-wide, weights ko-sliced for the early columns (w0 8x, w1 4x,
   w2 2x) alternating with x tiles at the work-per-byte-optimal ratio.
 - The PE schedule is a sequence of waves matched to slice arrival:
   wave(col0; m0..3) rides w0 quads at exactly the arrival rate (zero
   stalls, which also keeps the PE out of its low p-state), wave(col1;
   m0..3) rides w1 slices, wave(col2; m0..5) rides w2 halves; between
   waves, freshly landed x tiles are consumed against all resident
   columns. After the ~55us saturation point the PE never stalls again
   (w-pool depth 5 so no weight load waits on a buffer it can't have).
 - Bias add fused into the PSUM eviction on DVE; bias + stores ride the
   scalar (Act) HWDGE ring so they never queue behind loads. The last
   group is evicted in two 256-wide halves to shave the exposed tail.
"""

import sys

if "/opt/trn_rl_repo" not in sys.path:
    sys.path.insert(0, "/opt/trn_rl_repo")

import numpy as np

B, S, DIN, DOUT = 4, 4096, 4096, 4096
NCORES = 8
M_TOTAL = B * S              # 16384
M_LOC = M_TOTAL // NCORES    # 2048
P = 128
M_TILES = M_LOC // P         # 16 m-tiles of 128 rows
N_TILE = 512
N_TILES = DOUT // N_TILE     # 8
K_SUB = DIN // P             # 32 k-subtiles of 128
K_CHUNKS = K_SUB // 2        # 16 DoubleRow chunks of 256

_cached_nc = None


def _build():
    global _cached_nc
    if _cached_nc is not None:
        return _cached_nc

    import concourse.mybir as mybir
    import concourse.tile as tile
    from concourse import bacc

    nc = bacc.Bacc("TRN2", target_bir_lowering=False, debug=False,
                   num_devices=NCORES)

    # host-packed K-major tile blocks (see make_in_maps)
    xd = nc.dram_tensor("xd", [M_TILES, P, K_SUB, P], mybir.dt.float16,
                        kind="ExternalInput")
    wd = nc.dram_tensor("wd", [N_TILES, P, K_SUB, N_TILE], mybir.dt.float16,
                        kind="ExternalInput")
    bvec = nc.dram_tensor("bvec", [DOUT], mybir.dt.float16,
                          kind="ExternalInput")
    out = nc.dram_tensor("out", [M_LOC, DOUT], mybir.dt.float16,
                         kind="ExternalOutput")

    with tile.TileContext(nc) as tc:
        with tc.tile_pool(name="w8p", bufs=5) as w8p, \
             tc.tile_pool(name="x8p", bufs=1) as x8p, \
             tc.tile_pool(name="xst", bufs=2) as stp, \
             tc.tile_pool(name="outp", bufs=8) as outp, \
             tc.tile_pool(name="cst", bufs=1) as cst, \
             tc.tile_pool(name="psum", bufs=8, space="PSUM") as psump:

            # resident fp8 x: 16 tiles of [ki, ko, 128] e5m2
            x8 = [x8p.tile([P, K_SUB, P], mybir.dt.float8e5,
                           tag=f"x8_{m}", name=f"x8_{m}")
                  for m in range(M_TILES)]
            xst = {}
            w8 = {}

            # ---- scalar (Act) HWDGE ring: bias broadcast first ----
            bias_rep = cst.tile([P, DOUT], mybir.dt.float16)
            nc.scalar.dma_start(bias_rep[:],
                                bvec.ap()[None, :].to_broadcast((P, DOUT)))

            def load_x_hw(m, slices, eng):
                xst[m] = stp.tile([P, K_SUB, P], mybir.dt.float16,
                                  tag="xst", name=f"xst_{m}")
                return [eng.dma_start(xst[m][:, a:b, :], xd[m, :, a:b, :])
                        for a, b in slices]

            def cast_x(m, slices):
                return [nc.vector.tensor_copy(x8[m][:, a:b, :],
                                              xst[m][:, a:b, :])
                        for a, b in slices]

            # ---- sync HWDGE ring: only x0,x1 raw fp16, fetched during
            # SWDGE's startup dead-zone; the ring then goes silent so it
            # never preempts the weight stream (HWDGE has strict priority) --
            load_x_hw(0, [(0, 8), (8, 32)], nc.sync)
            load_x_hw(1, [(0, 16), (16, 32)], nc.sync)
            cast_x(0, [(0, 4), (4, 8), (8, 20), (20, 32)])
            cast_x(1, [(0, 16), (16, 32)])

            # ---- gpsimd SWDGE: single FIFO, greedy x:w balance (take
            # another x tile while m < 4*cols, else the next w column) ----
            def load_w(j, chunks):
                w8[j] = w8p.tile([P, K_SUB, N_TILE], mybir.dt.float8e4,
                                 tag="w8", name=f"w8_{j}")
                step = K_SUB // chunks
                for c in range(chunks):
                    ko = slice(c * step, (c + 1) * step)
                    nc.gpsimd.dma_start(w8[j][:, ko, :], wd[j, :, ko, :])

            def load_x_sw(m):
                nc.gpsimd.dma_start(x8[m][:], xd[m, :, :, :])

            w8[0] = w8p.tile([P, K_SUB, N_TILE], mybir.dt.float8e4,
                             tag="w8", name="w8_0")
            w8[1] = w8p.tile([P, K_SUB, N_TILE], mybir.dt.float8e4,
                             tag="w8", name="w8_1")

            def load_w_slice(j, a, b):
                return nc.gpsimd.dma_start(w8[j][:, a:b, :],
                                           wd[j, :, a:b, :])

            def load_x_slice(m, a, b):
                return nc.gpsimd.dma_start(x8[m][:, a:b, :],
                                           xd[m, :, a:b, :])

            load_x_slice(2, 0, 16)
            load_x_slice(3, 0, 16)
            for c in range(4):
                load_w_slice(0, 4 * c, 4 * c + 4)
            load_x_slice(2, 16, 32)
            load_x_slice(3, 16, 32)
            load_x_sw(4)
            for c in range(4, 8):
                load_w_slice(0, 4 * c, 4 * c + 4)
            load_w_slice(1, 0, 8)
            load_w_slice(1, 8, 16)
            load_x_sw(5)
            load_w_slice(1, 16, 24)
            load_w_slice(1, 24, 32)
            load_x_sw(6)
            load_x_sw(7)
            load_w(2, chunks=2)
            load_x_sw(8)
            load_x_sw(9)
            load_w(3, chunks=1)
            load_x_sw(10)
            load_x_sw(11)
            load_w(4, chunks=1)
            load_x_sw(12)
            load_x_sw(13)
            load_w(5, chunks=1)
            load_x_sw(14)
            load_x_sw(15)
            load_w(6, chunks=1)
            load_w(7, chunks=1)

            def mm(ps, j, m, kc):
                nc.tensor.matmul(
                    ps[:],
                    x8[m][:, 2 * kc:2 * kc + 2, :],
                    w8[j][:, 2 * kc:2 * kc + 2, :],
                    start=(kc == 0),
                    stop=(kc == K_CHUNKS - 1),
                    perf_mode=mybir.MatmulPerfMode.DoubleRow,
                )

            def evict(j, m, ps, split=False):
                ns = slice(j * N_TILE, (j + 1) * N_TILE)
                rows = slice(m * P, (m + 1) * P)
                if not split:
                    ob = outp.tile([P, N_TILE], mybir.dt.float16, tag="ob",
                                   name=f"ob_{j}_{m}")
                    nc.vector.tensor_add(ob[:], ps[:], bias_rep[:, ns])
                    nc.scalar.dma_start(out[rows, ns], ob[:])
                else:
                    h = N_TILE // 2
                    for half in range(2):
                        cs = slice(half * h, (half + 1) * h)
                        on = slice(j * N_TILE + half * h,
                                   j * N_TILE + (half + 1) * h)
                        ob = outp.tile([P, h], mybir.dt.float16, tag="obh",
                                       name=f"obh_{half}")
                        nc.vector.tensor_add(ob[:], ps[:, cs],
                                             bias_rep[:, on])
                        nc.scalar.dma_start(out[rows, on], ob[:])

            def do_group(j, m, split=False):
                ps = psump.tile([P, N_TILE], mybir.dt.float32, tag="ps",
                                name=f"ps_{j}_{m}")
                for kc in range(K_CHUNKS):
                    mm(ps, j, m, kc)
                evict(j, m, ps, split=split)

            def wave(j, ms, kc_run):
                """Interleaved chains over `ms`, kc emitted in runs of
                `kc_run` so each arriving w-slice unlocks a full-width run."""
                pss = {m: psump.tile([P, N_TILE], mybir.dt.float32, tag="ps",
                                     name=f"ps_{j}_{m}") for m in ms}
                for r0 in range(0, K_CHUNKS, kc_run):
                    for kc in range(r0, r0 + kc_run):
                        for m in ms:
                            mm(pss[m], j, m, kc)
                return pss

            # ---- phase A: col-0 wave over m0..3 riding w0 quads (x2,x3
            # prefetched ahead of w0, so consumption rate matches arrival) --
            psA = wave(0, [0, 1, 2, 3], kc_run=2)
            for m in range(4):
                evict(0, m, psA[m])

            # ---- phase B: (0,4) as x4 lands mid-w0 ----
            do_group(0, 4)

            # ---- phase C: col-1 wave over m0..3 riding w1 slices ----
            psC = wave(1, [0, 1, 2, 3], kc_run=4)
            for m in range(4):
                evict(1, m, psC[m])

            # ---- phase D: m4,m5 catch-up against cols 0,1 ----
            do_group(0, 5)
            do_group(1, 4)
            do_group(1, 5)

            # ---- phase E: col-2 wave over m0..5 riding w2 halves ----
            psE = wave(2, list(range(6)), kc_run=8)
            for m in range(6):
                evict(2, m, psE[m])

            # ---- phase F: m6,m7 against cols 0,1,2 ----
            for m in (6, 7):
                for j in range(3):
                    do_group(j, m)

            # ---- phase G: col-3 m0..7 (w3 fully resident by now) ----
            for m in range(8):
                do_group(3, m)

            # ---- phase H: m8..15 against cols 0..3 ----
            for m in range(8, M_TILES):
                for j in range(4):
                    do_group(j, m)

            # ---- phase I: columns 4..7 ----
            for j in range(4, N_TILES):
                for m in range(M_TILES):
                    last = (j == N_TILES - 1 and m == M_TILES - 1)
                    do_group(j, m, split=last)

    nc.compile()
    _cached_nc = nc
    return nc


def make_in_maps(x, weight, bias):
    x = np.asarray(x)
    weight = np.asarray(weight)
    bias = np.ascontiguousarray(np.asarray(bias))
    assert x.dtype == np.float16 and weight.dtype == np.float16

    # weight [DOUT, DIN] -> [j, ki, ko, n]: wd[j,ki,ko,n] = weight[j*512+n,
    # ko*128+ki] (i.e. weight.T in per-tile K-major blocks)
    wd = np.ascontiguousarray(
        weight.reshape(N_TILES, N_TILE, K_SUB, P).transpose(0, 3, 2, 1))

    xf = x.reshape(M_TOTAL, DIN)
    in_maps = []
    for c in range(NCORES):
        xc = xf[c * M_LOC:(c + 1) * M_LOC]
        # [M_LOC, DIN] -> [m-tile, ki, ko, m]: xd[t,ki,ko,m] = xc[t*128+m,
        # ko*128+ki]
        xd = np.ascontiguousarray(
            xc.reshape(M_TILES, P, K_SUB, P).transpose(0, 3, 2, 1))
        in_maps.append({"xd": xd, "wd": wd, "bvec": bias})
    return in_maps


def gather_out(results):
    out = np.concatenate([r["out"] for r in results], axis=0)
    return out.reshape(B, S, DOUT)


def kernel(x, weight, bias):
    from concourse.bass_utils import run_bass_kernel_spmd

    nc = _build()
    in_maps = make_in_maps(x, weight, bias)
    res = run_bass_kernel_spmd(nc, in_maps, core_ids=list(range(NCORES)))
    return gather_out(res.results)


# revision 32
# speedup vs baseline: 1.0145x; 1.0139x over previous
"""FP8 GEMM kernel for Trainium2 (8 NeuronCores, SPMD data-parallel over tokens).

Computes: out = fp16( fp32( e5m2(x) @ e4m3(weight.T) ) + bias )
  x      [4, 4096, 4096] fp16
  weight [4096, 4096]    fp16  (out_features, in_features)
  bias   [4096]          fp16
  out    [4, 4096, 4096] fp16

Sharding: token dim (B*S = 16384) split across 8 cores (2048 rows each);
weight + bias replicated. No collectives; host concatenates the outputs.

Layout: the host pre-packs both operands into per-tile K-major blocks
(`[tile][ki=128][ko=32][free]`) so every device load is fully contiguous.

Per-core pipeline (~478us vs the 442us PE floor of 2048 DoubleRow fp8
matmuls x 216ns; HBM read is ~390GB/s SHARED across HWDGE+SWDGE with
strict HWDGE priority, so the ramp rations it carefully):
 - Only x0,x1 arrive as raw fp16 on the sync HWDGE ring (during SWDGE's
   ~4us startup dead-zone) and are cast to e5m2 on DVE (bit-exact RNE,
   probed vs ml_dtypes). The ring then goes silent: any extra HWDGE
   traffic in the first ~100us preempts the weight stream and starves
   the PE (measured +25us).
 - Everything else is a single SWDGE cast-DMA FIFO in exact consumption
   order: x2,x3 first halves prefetched ahead of w0 so the col-0 wave
   can be 4-wide, weights ko-sliced for the early columns (w0 8x, w1 4x,
   w2 2x) alternating with x tiles at the work-per-byte-optimal ratio.
 - The PE schedule is a sequence of waves matched to slice arrival:
   wave(col0; m0..3) rides w0 quads at exactly the arrival rate (zero
   stalls, which also keeps the PE out of its low p-state), wave(col1;
   m0..3) rides w1 slices, wave(col2; m0..5) rides w2 halves; between
   waves, freshly landed x tiles are consumed against all resident
   columns. After the ~55us saturation point the PE never stalls again
   (w-pool depth 5 so no weight load waits on a buffer it can't have).
 - Bias add fused into the PSUM eviction on DVE; bias + stores ride the
   scalar (Act) HWDGE ring so they never queue behind loads. The last
   group is evicted in two 256-wide halves to shave the exposed tail.
"""

import sys

if "/opt/trn_rl_repo" not in sys.path:
    sys.path.insert(0, "/opt/trn_rl_repo")

import numpy as np

B, S, DIN, DOUT = 4, 4096, 4096, 4096
NCORES = 8
M_TOTAL = B * S              # 16384
M_LOC = M_TOTAL // NCORES    # 2048
P = 128
M_TILES = M_LOC // P         # 16 m-tiles of 128 rows
N_TILE = 512
N_TILES = DOUT // N_TILE     # 8
K_SUB = DIN // P             # 32 k-subtiles of 128
K_CHUNKS = K_SUB // 2        # 16 DoubleRow chunks of 256

_cached_nc = None


def _build():
    global _cached_nc
    if _cached_nc is not None:
        return _cached_nc

    import concourse.mybir as mybir
    import concourse.tile as tile
    from concourse import bacc

    nc = bacc.Bacc("TRN2", target_bir_lowering=False, debug=False,
                   num_devices=NCORES, enable_partition_id=False,
                   monotonic_sem_count=0)

    # host-packed K-major tile blocks (see make_in_maps)
    xd = nc.dram_tensor("xd", [M_TILES, P, K_SUB, P], mybir.dt.float16,
                        kind="ExternalInput")
    wd = nc.dram_tensor("wd", [N_TILES, P, K_SUB, N_TILE], mybir.dt.float16,
                        kind="ExternalInput")
    bvec = nc.dram_tensor("bvec", [DOUT], mybir.dt.float16,
                          kind="ExternalInput")
    out = nc.dram_tensor("out", [M_LOC, DOUT], mybir.dt.float16,
                         kind="ExternalOutput")

    with tile.TileContext(nc) as tc:
        with tc.tile_pool(name="w8p", bufs=5) as w8p, \
             tc.tile_pool(name="x8p", bufs=1) as x8p, \
             tc.tile_pool(name="xst", bufs=2) as stp, \
             tc.tile_pool(name="outp", bufs=8) as outp, \
             tc.tile_pool(name="cst", bufs=1) as cst, \
             tc.tile_pool(name="psum", bufs=8, space="PSUM") as psump:

            # resident fp8 x: 16 tiles of [ki, ko, 128] e5m2
            x8 = [x8p.tile([P, K_SUB, P], mybir.dt.float8e5,
                           tag=f"x8_{m}", name=f"x8_{m}")
                  for m in range(M_TILES)]
            xst = {}
            w8 = {}

            # ---- scalar (Act) HWDGE ring: bias broadcast first ----
            bias_rep = cst.tile([P, DOUT], mybir.dt.float16)
            nc.scalar.dma_start(bias_rep[:],
                                bvec.ap()[None, :].to_broadcast((P, DOUT)))

            def load_x_hw(m, slices, eng):
                xst[m] = stp.tile([P, K_SUB, P], mybir.dt.float16,
                                  tag="xst", name=f"xst_{m}")
                return [eng.dma_start(xst[m][:, a:b, :], xd[m, :, a:b, :])
                        for a, b in slices]

            def cast_x(m, slices):
                return [nc.vector.tensor_copy(x8[m][:, a:b, :],
                                              xst[m][:, a:b, :])
                        for a, b in slices]

            # ---- sync HWDGE ring: only x0,x1 raw fp16, fetched during
            # SWDGE's startup dead-zone; the ring then goes silent so it
            # never preempts the weight stream (HWDGE has strict priority) --
            load_x_hw(0, [(0, 8), (8, 32)], nc.sync)
            load_x_hw(1, [(0, 16), (16, 32)], nc.sync)
            cast_x(0, [(0, 4), (4, 8), (8, 20), (20, 32)])
            cast_x(1, [(0, 16), (16, 32)])

            # ---- gpsimd SWDGE: single FIFO, greedy x:w balance (take
            # another x tile while m < 4*cols, else the next w column) ----
            def load_w(j, chunks):
                w8[j] = w8p.tile([P, K_SUB, N_TILE], mybir.dt.float8e4,
                                 tag="w8", name=f"w8_{j}")
                step = K_SUB // chunks
                for c in range(chunks):
                    ko = slice(c * step, (c + 1) * step)
                    nc.gpsimd.dma_start(w8[j][:, ko, :], wd[j, :, ko, :])

            def load_x_sw(m):
                nc.gpsimd.dma_start(x8[m][:], xd[m, :, :, :])

            w8[0] = w8p.tile([P, K_SUB, N_TILE], mybir.dt.float8e4,
                             tag="w8", name="w8_0")
            w8[1] = w8p.tile([P, K_SUB, N_TILE], mybir.dt.float8e4,
                             tag="w8", name="w8_1")

            def load_w_slice(j, a, b):
                return nc.gpsimd.dma_start(w8[j][:, a:b, :],
                                           wd[j, :, a:b, :])

            def load_x_slice(m, a, b):
                return nc.gpsimd.dma_start(x8[m][:, a:b, :],
                                           xd[m, :, a:b, :])

            load_x_slice(2, 0, 16)
            load_x_slice(3, 0, 16)
            for c in range(4):
                load_w_slice(0, 4 * c, 4 * c + 4)
            load_x_slice(2, 16, 32)
            load_x_slice(3, 16, 32)
            load_x_sw(4)
            for c in range(4, 8):
                load_w_slice(0, 4 * c, 4 * c + 4)
            load_w_slice(1, 0, 8)
            load_w_slice(1, 8, 16)
            load_x_sw(5)
            load_w_slice(1, 16, 24)
            load_w_slice(1, 24, 32)
            load_x_sw(6)
            load_x_sw(7)
            load_w(2, chunks=2)
            load_x_sw(8)
            load_x_sw(9)
            load_w(3, chunks=1)
            load_x_sw(10)
            load_x_sw(11)
            load_w(4, chunks=1)
            load_x_sw(12)
            load_x_sw(13)
            load_w(5, chunks=1)
            load_x_sw(14)
            load_x_sw(15)
            load_w(6, chunks=1)
            load_w(7, chunks=1)

            def mm(ps, j, m, kc):
                nc.tensor.matmul(
                    ps[:],
                    x8[m][:, 2 * kc:2 * kc + 2, :],
                    w8[j][:, 2 * kc:2 * kc + 2, :],
                    start=(kc == 0),
                    stop=(kc == K_CHUNKS - 1),
                    perf_mode=mybir.MatmulPerfMode.DoubleRow,
                )

            def evict(j, m, ps, split=False):
                ns = slice(j * N_TILE, (j + 1) * N_TILE)
                rows = slice(m * P, (m + 1) * P)
                if not split:
                    ob = outp.tile([P, N_TILE], mybir.dt.float16, tag="ob",
                                   name=f"ob_{j}_{m}")
                    nc.vector.tensor_add(ob[:], ps[:], bias_rep[:, ns])
                    nc.scalar.dma_start(out[rows, ns], ob[:])
                else:
                    h = N_TILE // 2
                    for half in range(2):
                        cs = slice(half * h, (half + 1) * h)
                        on = slice(j * N_TILE + half * h,
                                   j * N_TILE + (half + 1) * h)
                        ob = outp.tile([P, h], mybir.dt.float16, tag="obh",
                                       name=f"obh_{half}")
                        nc.vector.tensor_add(ob[:], ps[:, cs],
                                             bias_rep[:, on])
                        nc.scalar.dma_start(out[rows, on], ob[:])

            def do_group(j, m, split=False):
                ps = psump.tile([P, N_TILE], mybir.dt.float32, tag="ps",
                                name=f"ps_{j}_{m}")
                for kc in range(K_CHUNKS):
                    mm(ps, j, m, kc)
                evict(j, m, ps, split=split)

            def wave(j, ms, kc_run):
                """Interleaved chains over `ms`, kc emitted in runs of
                `kc_run` so each arriving w-slice unlocks a full-width run."""
                pss = {m: psump.tile([P, N_TILE], mybir.dt.float32, tag="ps",
                                     name=f"ps_{j}_{m}") for m in ms}
                for r0 in range(0, K_CHUNKS, kc_run):
                    for kc in range(r0, r0 + kc_run):
                        for m in ms:
                            mm(pss[m], j, m, kc)
                return pss

            # ---- phase A: col-0 wave over m0..3 riding w0 quads (x2,x3
            # prefetched ahead of w0, so consumption rate matches arrival) --
            psA = wave(0, [0, 1, 2, 3], kc_run=2)
            for m in range(4):
                evict(0, m, psA[m])

            # ---- phase B: (0,4) as x4 lands mid-w0 ----
            do_group(0, 4)

            # ---- phase C: col-1 wave over m0..3 riding w1 slices ----
            psC = wave(1, [0, 1, 2, 3], kc_run=4)
            for m in range(4):
                evict(1, m, psC[m])

            # ---- phase D: m4,m5 catch-up against cols 0,1 ----
            do_group(0, 5)
            do_group(1, 4)
            do_group(1, 5)

            # ---- phase E: col-2 wave over m0..5 riding w2 halves ----
            psE = wave(2, list(range(6)), kc_run=8)
            for m in range(6):
                evict(2, m, psE[m])

            # ---- phase F: m6,m7 against cols 0,1,2 ----
            for m in (6, 7):
                for j in range(3):
                    do_group(j, m)

            # ---- phase G: col-3 m0..7 (w3 fully resident by now) ----
            for m in range(8):
                do_group(3, m)

            # ---- phase H: m8..15 against cols 0..3 ----
            for m in range(8, M_TILES):
                for j in range(4):
                    do_group(j, m)

            # ---- phase I: columns 4..7 ----
            for j in range(4, N_TILES):
                for m in range(M_TILES):
                    last = (j == N_TILES - 1 and m == M_TILES - 1)
                    do_group(j, m, split=last)

    nc.compile()
    _cached_nc = nc
    return nc


def make_in_maps(x, weight, bias):
    x = np.asarray(x)
    weight = np.asarray(weight)
    bias = np.ascontiguousarray(np.asarray(bias))
    assert x.dtype == np.float16 and weight.dtype == np.float16

    # weight [DOUT, DIN] -> [j, ki, ko, n]: wd[j,ki,ko,n] = weight[j*512+n,
    # ko*128+ki] (i.e. weight.T in per-tile K-major blocks)
    wd = np.ascontiguousarray(
        weight.reshape(N_TILES, N_TILE, K_SUB, P).transpose(0, 3, 2, 1))

    xf = x.reshape(M_TOTAL, DIN)
    in_maps = []
    for c in range(NCORES):
        xc = xf[c * M_LOC:(c + 1) * M_LOC]
        # [M_LOC, DIN] -> [m-tile, ki, ko, m]: xd[t,ki,ko,m] = xc[t*128+m,
        # ko*128+ki]
        xd = np.ascontiguousarray(
            xc.reshape(M_TILES, P, K_SUB, P).transpose(0, 3, 2, 1))
        in_maps.append({"xd": xd, "wd": wd, "bvec": bias})
    return in_maps


def gather_out(results):
    out = np.concatenate([r["out"] for r in results], axis=0)
    return out.reshape(B, S, DOUT)


def kernel(x, weight, bias):
    from concourse.bass_utils import run_bass_kernel_spmd

    nc = _build()
    in_maps = make_in_maps(x, weight, bias)
    res = run_bass_kernel_spmd(nc, in_maps, core_ids=list(range(NCORES)))
    return gather_out(res.results)
